# revision 1
# baseline (speedup 1.0000x reference)
"""GRUAggregation1d Trainium2 kernel.

Computes, for xs [B=16, 512, L=8192], z_prev [B, 128, L] (all fp32):
    q  = sigmoid(Wq@xs + Uq@z + bq)        (per position l, batch b)
    r  = sigmoid(Wr@xs + Ur@z + br)
    zt = tanh(Wz@xs + Uz@(r*z) + bz)
    out = q*z + (1-q)*zt

Sharding: data-parallel over batch. 8 cores x 2 batches each; weights
replicated. Each core loops over 2 batches x 16 position-tiles of 512.
Per tile: 15 matmuls (3 gates x (4 K-chunks of W + 1 U matmul)) accumulated
in PSUM, sigmoid/tanh on ScalarE (bias fused), gate combine on VectorE.
One-iteration software pipeline: the Uz@(r*z) matmul + tanh + combine of
tile i are emitted during tile i+1 so the PE never stalls on the
r -> r*z dependency chain.

Matmul inputs are bf16 (xs and the weights are cast on the host, halving
the xs DMA; z_prev is DMA'd fp32 and cast to bf16 on ScalarE so the final
combine q*z + (1-q)*zt still sees fp32 z). PSUM accumulation is fp32.
"""

from contextlib import ExitStack

import ml_dtypes
import numpy as np

import concourse.bass as bass
import concourse.mybir as mybir
import concourse.tile as tile
from concourse import bacc
from concourse.bass_utils import run_bass_kernel_spmd

B, IN_DIM, WIDTH, L = 16, 512, 128, 8192
N_CORES = 8
B_PER = B // N_CORES          # batches per core
KC = IN_DIM // 128            # K chunks for the W matmuls
NT = 512                      # positions per tile
N_LT = L // NT                # position tiles per batch
F32 = mybir.dt.float32
BF16 = mybir.dt.bfloat16

_module_cache = {}


def _build():
    key = ("bf16", NT)
    if key in _module_cache:
        return _module_cache[key]

    nc = bacc.Bacc("TRN2", target_bir_lowering=False, debug=False,
                   num_devices=N_CORES)

    xs_d = nc.dram_tensor("xs", [B_PER, IN_DIM, L], BF16, kind="ExternalInput").ap()
    zp_d = nc.dram_tensor("zp", [B_PER, WIDTH, L], F32, kind="ExternalInput").ap()
    w_d = {}
    for g in ("q", "r", "z"):
        w_d[f"w{g}"] = nc.dram_tensor(f"w{g}", [128, KC, 128], BF16,
                                      kind="ExternalInput").ap()
        w_d[f"u{g}"] = nc.dram_tensor(f"u{g}", [128, 128], BF16,
                                      kind="ExternalInput").ap()
        w_d[f"b{g}"] = nc.dram_tensor(f"b{g}", [128, 1], F32,
                                      kind="ExternalInput").ap()
    out_d = nc.dram_tensor("out", [B_PER, WIDTH, L], F32, kind="ExternalOutput").ap()

    # [b, (k p), l] -> [b, p, k, l] so a position-tile slice is a [128, KC, NT]
    # DMA with 1KB contiguous rows
    xs_r = xs_d.rearrange("b (k p) l -> b p k l", p=128)

    with tile.TileContext(nc) as tc, ExitStack() as ctx:
        wpool = ctx.enter_context(tc.tile_pool(name="weights", bufs=1))
        io = ctx.enter_context(tc.tile_pool(name="io", bufs=3))
        acts = ctx.enter_context(tc.tile_pool(name="acts", bufs=3))
        psum = ctx.enter_context(tc.tile_pool(name="psum", bufs=2, space="PSUM"))

        # weights + biases, loaded once
        w_sb = {}
        for g in ("q", "r", "z"):
            wt = wpool.tile([128, KC, 128], BF16, tag=f"w{g}")
            nc.sync.dma_start(wt[:], w_d[f"w{g}"][:])
            ut = wpool.tile([128, 128], BF16, tag=f"u{g}")
            nc.sync.dma_start(ut[:], w_d[f"u{g}"][:])
            bt = wpool.tile([128, 1], F32, tag=f"b{g}")
            nc.sync.dma_start(bt[:], w_d[f"b{g}"][:])
            w_sb[g] = (wt, ut, bt)

        # one software-pipeline stage of carried state per tile:
        # (zt_psum, rz, q_sbuf, z_sbuf, out_slice)
        carry = None

        def finish_prev(carry):
            """Emit the trailing half of tile i (Uz matmul, tanh, combine,
            store) -- called while tile i+1's leading half is in flight."""
            zt_ps, rz, q_s, z_t, out_slice = carry
            _, uz_t, bz_t = w_sb["z"]
            nc.tensor.matmul(zt_ps[:], uz_t[:], rz[:], start=False, stop=True)
            zt_s = acts.tile([128, NT], F32, tag="zt_s")
            nc.scalar.activation(zt_s[:], zt_ps[:],
                                 mybir.ActivationFunctionType.Tanh, bias=bz_t[:])
            # out = zt + q*(z - zt)
            diff = acts.tile([128, NT], F32, tag="diff")
            nc.vector.tensor_sub(diff[:], z_t[:], zt_s[:])
            prod = acts.tile([128, NT], F32, tag="prod")
            nc.vector.tensor_mul(prod[:], q_s[:], diff[:])
            o_t = acts.tile([128, NT], F32, tag="o_t")
            nc.vector.tensor_add(o_t[:], zt_s[:], prod[:])
            nc.sync.dma_start(out_slice, o_t[:])

        for b in range(B_PER):
            for i in range(N_LT):
                l0 = i * NT
                xs_t = io.tile([128, KC, NT], BF16, tag="xs_t")
                nc.sync.dma_start(xs_t[:], xs_r[b][:, :, l0:l0 + NT])
                z_t = io.tile([128, NT], F32, tag="z_t")
                nc.sync.dma_start(z_t[:], zp_d[b][:, l0:l0 + NT])
                # bf16 copy of z for the U matmuls (ScalarE has spare cycles)
                z_bf = io.tile([128, NT], BF16, tag="z_bf")
                nc.scalar.activation(z_bf[:], z_t[:],
                                     mybir.ActivationFunctionType.Copy)

                if carry is not None:
                    finish_prev(carry)
                    carry = None

                # ---- q gate ----
                wq_t, uq_t, bq_t = w_sb["q"]
                q_ps = psum.tile([128, NT], F32, tag="q_ps")
                for k in range(KC):
                    nc.tensor.matmul(q_ps[:], wq_t[:, k, :], xs_t[:, k, :],
                                     start=(k == 0), stop=False)
                nc.tensor.matmul(q_ps[:], uq_t[:], z_bf[:], start=False, stop=True)
                q_s = acts.tile([128, NT], F32, tag="q_s")
                nc.scalar.activation(q_s[:], q_ps[:],
                                     mybir.ActivationFunctionType.Sigmoid,
                                     bias=bq_t[:])

                # ---- r gate ----
                wr_t, ur_t, br_t = w_sb["r"]
                r_ps = psum.tile([128, NT], F32, tag="r_ps")
                for k in range(KC):
                    nc.tensor.matmul(r_ps[:], wr_t[:, k, :], xs_t[:, k, :],
                                     start=(k == 0), stop=False)
                nc.tensor.matmul(r_ps[:], ur_t[:], z_bf[:], start=False, stop=True)
                r_s = acts.tile([128, NT], BF16, tag="r_s")
                nc.scalar.activation(r_s[:], r_ps[:],
                                     mybir.ActivationFunctionType.Sigmoid,
                                     bias=br_t[:])

                # ---- zt: W part only; Uz@(r*z) lands next iteration ----
                wz_t, _, _ = w_sb["z"]
                zt_ps = psum.tile([128, NT], F32, tag="zt_ps")
                for k in range(KC):
                    nc.tensor.matmul(zt_ps[:], wz_t[:, k, :], xs_t[:, k, :],
                                     start=(k == 0), stop=False)

                rz = acts.tile([128, NT], BF16, tag="rz")
                nc.vector.tensor_mul(rz[:], r_s[:], z_bf[:])
                carry = (zt_ps, rz, q_s, z_t, out_d[b][:, l0:l0 + NT])

        finish_prev(carry)

    nc.compile()
    _module_cache[key] = nc
    return nc


def _pack_w(w):
    # W [128 out, 512 in] -> [128 part=in%128, KC, 128 out]
    return np.ascontiguousarray(
        w.T.reshape(KC, 128, 128).transpose(1, 0, 2)).astype(ml_dtypes.bfloat16)


def _run(inputs, trace=False, **run_kwargs):
    xs = np.asarray(inputs["xs"], dtype=np.float32)
    zp = np.ascontiguousarray(np.asarray(inputs["z_prev"], dtype=np.float32))
    assert xs.shape == (B, IN_DIM, L) and zp.shape == (B, WIDTH, L)
    xs_bf = np.ascontiguousarray(xs.astype(ml_dtypes.bfloat16))

    packed = {}
    for g, (wn, un, wbn, ubn) in {
        "q": ("Wq_w", "Uq_w", "Wq_b", "Uq_b"),
        "r": ("Wr_w", "Ur_w", "Wr_b", "Ur_b"),
        "z": ("Wz_w", "Uz_w", "Wz_b", "Uz_b"),
    }.items():
        packed[f"w{g}"] = _pack_w(np.asarray(inputs[wn], dtype=np.float32))
        packed[f"u{g}"] = np.ascontiguousarray(
            np.asarray(inputs[un], dtype=np.float32).T.astype(ml_dtypes.bfloat16))
        packed[f"b{g}"] = np.ascontiguousarray(
            (np.asarray(inputs[wbn], dtype=np.float32)
             + np.asarray(inputs[ubn], dtype=np.float32)).reshape(128, 1))

    nc = _build()
    in_maps = []
    for c in range(N_CORES):
        m = {"xs": np.ascontiguousarray(xs_bf[c * B_PER:(c + 1) * B_PER]),
             "zp": np.ascontiguousarray(zp[c * B_PER:(c + 1) * B_PER])}
        m.update(packed)
        in_maps.append(m)

    res = run_bass_kernel_spmd(nc, in_maps, core_ids=list(range(N_CORES)),
                               trace=trace, **run_kwargs)
    out = np.concatenate([res.results[c]["out"] for c in range(N_CORES)], axis=0)
    return out, res


def kernel(**inputs):
    out, _ = _run(inputs, trace=False)
    return out



# revision 4
# speedup vs baseline: 1.1805x; 1.1805x over previous
"""GRUAggregation1d Trainium2 kernel.

Computes, for xs [B=16, 512, L=8192], z_prev [B, 128, L] (all fp32):
    q  = sigmoid(Wq@xs + Uq@z + bq)        (per position l, batch b)
    r  = sigmoid(Wr@xs + Ur@z + br)
    zt = tanh(Wz@xs + Uz@(r*z) + bz)
    out = q*z + (1-q)*zt

Sharding: data-parallel over batch, 8 cores x 2 batches. Per core: 32
position-tiles of 512. Matmuls are bf16 (fp8 DoubleRow was tried and
rejected: e4m3 noise on the uniformly-distributed weights alone is
~2.9e-2 max-rel error, and the residual-correction matmuls that would
fix it double the LDWEIGHTS volume, which does not hide under 107ns
DoubleRow matmuls).

vs the original baseline (157us):
  - DMA diet: z_prev and the output travel as bf16 (host casts), so
    per-core traffic is 16 MiB xs + 4 z + 4 out instead of 16+8+8.
  - DMA batching: host packs xs+z into one row per (partition, tile)
    (4096B bf16 xs + 1024B bf16 z); ONE input DMA per 2 tiles (10KB
    rows) and one output DMA per 2 tiles instead of 3 DMAs per tile.
    This amortizes the ~625ns HWDGE descriptor-generation serialization
    and the SP-sequencer trigger cost (~565ns each).
  - All elementwise work is bf16: the z->bf16 ScalarE cast disappears,
    and the 4 VectorE ops per tile (rz, and the 3-op combine
    out = zt + q*(z-zt)) run in the DVE 2x mode.
  - Weights packed into a single [128, 15, 128] bf16 tensor + [128,3]
    f32 bias tensor: 2 weight DMAs total.
"""

from contextlib import ExitStack

import ml_dtypes
import numpy as np

import concourse.bass as bass
import concourse.mybir as mybir
import concourse.tile as tile
from concourse import bacc
from concourse.bass_utils import run_bass_kernel_spmd

B, IN_DIM, WIDTH, L = 16, 512, 128, 8192
N_CORES = 8
B_PER = B // N_CORES          # batches per core
KC = IN_DIM // 128            # K chunks for the W matmuls
NT = 512                      # positions per tile
N_LT = L // NT                # position tiles per batch
T = B_PER * N_LT              # tiles per core
SS = 2                        # tiles per DMA superstep
XS_B = KC * NT * 2            # bf16 xs bytes per row
Z_B = 2 * NT                  # bf16 z bytes per row
ROW = XS_B + Z_B              # input row bytes per (partition, tile)

F32 = mybir.dt.float32
BF16 = mybir.dt.bfloat16
U8 = mybir.dt.uint8

_module_cache = {}


def _build():
    key = ("bf16v3", NT, SS)
    if key in _module_cache:
        return _module_cache[key]

    nc = bacc.Bacc("TRN2", target_bir_lowering=False, debug=False,
                   num_devices=N_CORES)

    inp_d = nc.dram_tensor("inp", [128, T, ROW], U8, kind="ExternalInput").ap()
    w_d = nc.dram_tensor("wall", [128, 15, 128], BF16,
                         kind="ExternalInput").ap()
    b_d = nc.dram_tensor("ball", [128, 3], F32, kind="ExternalInput").ap()
    out_d = nc.dram_tensor("out", [128, T, NT], BF16,
                           kind="ExternalOutput").ap()

    with tile.TileContext(nc) as tc, ExitStack() as ctx:
        wpool = ctx.enter_context(tc.tile_pool(name="weights", bufs=1))
        io = ctx.enter_context(tc.tile_pool(name="io", bufs=4))
        sig = ctx.enter_context(tc.tile_pool(name="sig", bufs=3))
        acts = ctx.enter_context(tc.tile_pool(name="acts", bufs=2))
        ost_p = ctx.enter_context(tc.tile_pool(name="ost", bufs=3))
        ps_q = ctx.enter_context(tc.tile_pool(name="ps_q", bufs=2,
                                              space="PSUM"))
        ps_r = ctx.enter_context(tc.tile_pool(name="ps_r", bufs=2,
                                              space="PSUM"))
        ps_zt = ctx.enter_context(tc.tile_pool(name="ps_zt", bufs=3,
                                               space="PSUM"))

        wall = wpool.tile([128, 15, 128], BF16, tag="wall")
        nc.sync.dma_start(wall[:], w_d[:])
        ball = wpool.tile([128, 3], F32, tag="ball")
        nc.sync.dma_start(ball[:], b_d[:])

        carry = None
        ost = [None]

        def finish(c):
            """Trailing half of tile tc_ (Uz@(rz) matmul, tanh, combine,
            store), emitted during tile tc_+1."""
            zt_ps, q_s, rz, z_v, tc_ = c
            nc.tensor.matmul(zt_ps[:], wall[:, 14, :], rz[:],
                             start=False, stop=True)
            zt_s = acts.tile([128, NT], BF16, tag="zt_s")
            nc.scalar.activation(zt_s[:], zt_ps[:],
                                 mybir.ActivationFunctionType.Tanh,
                                 bias=ball[:, 2:3])
            sc, jc = divmod(tc_, SS)
            if jc == 0:
                ost[0] = ost_p.tile([128, SS, NT], BF16, tag="ost",
                                    name="ost")
            # out = zt + q*(z - zt)
            diff = acts.tile([128, NT], BF16, tag="diff")
            nc.vector.tensor_sub(diff[:], z_v, zt_s[:])
            prod = acts.tile([128, NT], BF16, tag="prod")
            nc.vector.tensor_mul(prod[:], q_s[:], diff[:])
            nc.vector.tensor_add(ost[0][:, jc, :], zt_s[:], prod[:])
            if jc == SS - 1:
                nc.sync.dma_start(out_d[:, SS * sc:SS * (sc + 1), :],
                                  ost[0][:])

        io_t = None
        for t in range(T):
            s, j = divmod(t, SS)
            if j == 0:
                io_t = io.tile([128, SS, ROW], U8, tag="io")
                nc.sync.dma_start(io_t[:], inp_d[:, SS * s:SS * (s + 1), :])
            xs_v = io_t[:, j, 0:XS_B].bitcast(BF16).rearrange(
                "p (k n) -> p k n", k=KC)
            z_v = io_t[:, j, XS_B:ROW].bitcast(BF16)

            q_ps = ps_q.tile([128, NT], F32, tag="q")
            for k in range(KC):
                nc.tensor.matmul(q_ps[:], wall[:, k, :], xs_v[:, k, :],
                                 start=(k == 0), stop=False)
            nc.tensor.matmul(q_ps[:], wall[:, 4, :], z_v,
                             start=False, stop=True)
            r_ps = ps_r.tile([128, NT], F32, tag="r")
            for k in range(KC):
                nc.tensor.matmul(r_ps[:], wall[:, 5 + k, :], xs_v[:, k, :],
                                 start=(k == 0), stop=False)
            nc.tensor.matmul(r_ps[:], wall[:, 9, :], z_v,
                             start=False, stop=True)

            if carry is not None:
                finish(carry)
                carry = None

            zt_ps = ps_zt.tile([128, NT], F32, tag="zt")
            for k in range(KC):
                nc.tensor.matmul(zt_ps[:], wall[:, 10 + k, :], xs_v[:, k, :],
                                 start=(k == 0), stop=False)

            q_s = sig.tile([128, NT], BF16, tag="q_s")
            nc.scalar.activation(q_s[:], q_ps[:],
                                 mybir.ActivationFunctionType.Sigmoid,
                                 bias=ball[:, 0:1])
            r_s = acts.tile([128, NT], BF16, tag="r_s")
            nc.scalar.activation(r_s[:], r_ps[:],
                                 mybir.ActivationFunctionType.Sigmoid,
                                 bias=ball[:, 1:2])
            rz = acts.tile([128, NT], BF16, tag="rz")
            nc.vector.tensor_mul(rz[:], r_s[:], z_v)

            carry = (zt_ps, q_s, rz, z_v, t)

        finish(carry)

    nc.compile()
    _module_cache[key] = nc
    return nc


def _pack_inputs(xs, zp):
    """Per-core [128, T, ROW] uint8 rows: 4096B bf16 xs + 1024B bf16 z."""
    xsb = xs.astype(ml_dtypes.bfloat16)
    v = xsb.reshape(B, KC, 128, N_LT, NT)        # [b, k, p, i, n]
    v = np.ascontiguousarray(v.transpose(0, 2, 3, 1, 4))  # [b, p, i, k, n]
    v = v.reshape(B, 128, N_LT, KC * NT).view(np.uint8)   # [b, p, i, XS_B]
    zb = zp.astype(ml_dtypes.bfloat16).reshape(B, 128, N_LT, NT)
    zb = zb.view(np.uint8).reshape(B, 128, N_LT, Z_B)
    rows = np.concatenate([v, zb], axis=-1)      # [b, p, i, ROW]
    cores = []
    for c in range(N_CORES):
        rc = rows[c * B_PER:(c + 1) * B_PER]     # [B_PER, p, i, ROW]
        rc = rc.transpose(1, 0, 2, 3).reshape(128, T, ROW)
        cores.append(np.ascontiguousarray(rc))
    return cores


def _pack_weights(inputs):
    wall = np.zeros((128, 15, 128), dtype=np.float32)
    ball = np.zeros((128, 3), dtype=np.float32)
    for g, (wn, un, wbn, ubn) in enumerate((
            ("Wq_w", "Uq_w", "Wq_b", "Uq_b"),
            ("Wr_w", "Ur_w", "Wr_b", "Ur_b"),
            ("Wz_w", "Uz_w", "Wz_b", "Uz_b"))):
        W = np.asarray(inputs[wn], dtype=np.float32)         # [128, 512]
        U = np.asarray(inputs[un], dtype=np.float32)         # [128, 128]
        Wt = W.T.reshape(KC, 128, 128)                       # [k, p, m]
        for k in range(KC):
            wall[:, 5 * g + k, :] = Wt[k]
        wall[:, 5 * g + 4, :] = U.T
        ball[:, g] = (np.asarray(inputs[wbn], dtype=np.float32)
                      + np.asarray(inputs[ubn], dtype=np.float32))
    wall_b = np.ascontiguousarray(wall.astype(ml_dtypes.bfloat16))
    return wall_b, np.ascontiguousarray(ball)


def _run(inputs, trace=False, **run_kwargs):
    xs = np.asarray(inputs["xs"], dtype=np.float32)
    zp = np.ascontiguousarray(np.asarray(inputs["z_prev"], dtype=np.float32))
    assert xs.shape == (B, IN_DIM, L) and zp.shape == (B, WIDTH, L)

    inp_cores = _pack_inputs(xs, zp)
    wall, ball = _pack_weights(inputs)

    nc = _build()
    in_maps = [{"inp": inp_cores[c], "wall": wall, "ball": ball}
               for c in range(N_CORES)]

    res = run_bass_kernel_spmd(nc, in_maps, core_ids=list(range(N_CORES)),
                               trace=trace, **run_kwargs)
    outs = []
    for c in range(N_CORES):
        o = res.results[c]["out"]                 # [128, T, NT] bf16
        o = np.asarray(o).reshape(128, B_PER, N_LT, NT)
        o = o.transpose(1, 0, 2, 3).reshape(B_PER, 128, L)
        outs.append(o.astype(np.float32))
    out = np.concatenate(outs, axis=0)
    return out, res


def kernel(**inputs):
    out, _ = _run(inputs, trace=False)
    return out


# revision 7
# speedup vs baseline: 1.2301x; 1.0420x over previous
"""GRUAggregation1d Trainium2 kernel.

Computes, for xs [B=16, 512, L=8192], z_prev [B, 128, L] (all fp32):
    q  = sigmoid(Wq@xs + Uq@z + bq)        (per position l, batch b)
    r  = sigmoid(Wr@xs + Ur@z + br)
    zt = tanh(Wz@xs + Uz@(r*z) + bz)
    out = q*z + (1-q)*zt

Sharding: data-parallel over batch, 8 cores x 2 batches. Per core: 32
position-tiles of 512. Matmuls are bf16 (fp8 DoubleRow was tried and
rejected: e4m3 noise on the uniformly-distributed weights alone is
~2.9e-2 max-rel error, and the residual-correction matmuls that would
fix it double the LDWEIGHTS volume, which does not hide under 107ns
DoubleRow matmuls).

vs the original baseline (157us):
  - DMA diet: z_prev and the output travel as bf16 (host casts), so
    per-core traffic is 16 MiB xs + 4 z + 4 out instead of 16+8+8.
  - DMA batching: host packs xs+z into one row per (partition, tile)
    (4096B bf16 xs + 1024B bf16 z); ONE input DMA per 2 tiles (10KB
    rows) and one output DMA per 2 tiles instead of 3 DMAs per tile.
    This amortizes the ~625ns HWDGE descriptor-generation serialization
    and the SP-sequencer trigger cost (~565ns each).
  - All elementwise work is bf16: the z->bf16 ScalarE cast disappears,
    and the 4 VectorE ops per tile (rz, and the 3-op combine
    out = zt + q*(z-zt)) run in the DVE 2x mode.
  - Weights packed into a single [128, 15, 128] bf16 tensor + [128,3]
    f32 bias tensor: 2 weight DMAs total.
"""

from contextlib import ExitStack

import ml_dtypes
import numpy as np

import concourse.bass as bass
import concourse.mybir as mybir
import concourse.tile as tile
from concourse import bacc
from concourse.bass_utils import run_bass_kernel_spmd

B, IN_DIM, WIDTH, L = 16, 512, 128, 8192
N_CORES = 8
B_PER = B // N_CORES          # batches per core
KC = IN_DIM // 128            # K chunks for the W matmuls
NT = 512                      # positions per tile
N_LT = L // NT                # position tiles per batch
T = B_PER * N_LT              # tiles per core
SS = 2                        # tiles per DMA superstep
XS_B = KC * NT * 2            # bf16 xs bytes per row
Z_B = 2 * NT                  # bf16 z bytes per row
ROW = XS_B + Z_B              # input row bytes per (partition, tile)

F32 = mybir.dt.float32
BF16 = mybir.dt.bfloat16
U8 = mybir.dt.uint8

_module_cache = {}


def _build():
    key = ("bf16v3", NT, SS)
    if key in _module_cache:
        return _module_cache[key]

    nc = bacc.Bacc("TRN2", target_bir_lowering=False, debug=False,
                   num_devices=N_CORES)

    inp_d = nc.dram_tensor("inp", [128, T, ROW], U8, kind="ExternalInput").ap()
    w_d = nc.dram_tensor("wall", [128, 15, 128], BF16,
                         kind="ExternalInput").ap()
    b_d = nc.dram_tensor("ball", [128, 3], F32, kind="ExternalInput").ap()
    out_d = nc.dram_tensor("out", [128, T, NT], BF16,
                           kind="ExternalOutput").ap()

    with tile.TileContext(nc) as tc, ExitStack() as ctx:
        wpool = ctx.enter_context(tc.tile_pool(name="weights", bufs=1))
        io = ctx.enter_context(tc.tile_pool(name="io", bufs=4))
        sig = ctx.enter_context(tc.tile_pool(name="sig", bufs=3))
        acts = ctx.enter_context(tc.tile_pool(name="acts", bufs=2))
        ost_p = ctx.enter_context(tc.tile_pool(name="ost", bufs=3))
        ps_q = ctx.enter_context(tc.tile_pool(name="ps_q", bufs=2,
                                              space="PSUM"))
        ps_r = ctx.enter_context(tc.tile_pool(name="ps_r", bufs=2,
                                              space="PSUM"))
        ps_zt = ctx.enter_context(tc.tile_pool(name="ps_zt", bufs=3,
                                               space="PSUM"))

        # first two input tiles land as single-tile DMAs so the PE can
        # start ~2us earlier; weights ride behind the first tile.
        warm = []
        for t in range(2):
            wt = io.tile([128, 1, ROW], U8, tag="io_w", name="warm")
            nc.sync.dma_start(wt[:], inp_d[:, t:t + 1, :])
            warm.append(wt)
            if t == 0:
                wall = wpool.tile([128, 15, 128], BF16, tag="wall")
                nc.sync.dma_start(wall[:], w_d[:])
                ball = wpool.tile([128, 3], F32, tag="ball")
                nc.sync.dma_start(ball[:], b_d[:])

        carry = None
        ost = [None]

        def finish(c):
            """Trailing half of tile tc_ (Uz@(rz) matmul, tanh, combine,
            store), emitted during tile tc_+1."""
            zt_ps, q_s, rz, z_v, tc_ = c
            nc.tensor.matmul(zt_ps[:], wall[:, 14, :], rz[:],
                             start=False, stop=True)
            zt_s = acts.tile([128, NT], BF16, tag="zt_s")
            nc.scalar.activation(zt_s[:], zt_ps[:],
                                 mybir.ActivationFunctionType.Tanh,
                                 bias=ball[:, 2:3])
            sc, jc = divmod(tc_, SS)
            if jc == 0:
                ost[0] = ost_p.tile([128, SS, NT], BF16, tag="ost",
                                    name="ost")
            # out = zt + q*(z - zt)
            diff = acts.tile([128, NT], BF16, tag="diff")
            nc.vector.tensor_sub(diff[:], z_v, zt_s[:])
            prod = acts.tile([128, NT], BF16, tag="prod")
            nc.vector.tensor_mul(prod[:], q_s[:], diff[:])
            nc.vector.tensor_add(ost[0][:, jc, :], zt_s[:], prod[:])
            if jc == SS - 1:
                # out-DMAs ride the Activation HWDGE queue so the SP queue
                # only carries input DMAs
                nc.scalar.dma_start(out_d[:, SS * sc:SS * (sc + 1), :],
                                    ost[0][:])

        io_t = None
        for t in range(T):
            s, j = divmod(t, SS)
            if t < 2:
                cur, cj = warm[t], 0
            else:
                if j == 0:
                    io_t = io.tile([128, SS, ROW], U8, tag="io")
                    nc.sync.dma_start(io_t[:],
                                      inp_d[:, SS * s:SS * (s + 1), :])
                cur, cj = io_t, j
            xs_v = cur[:, cj, 0:XS_B].bitcast(BF16).rearrange(
                "p (k n) -> p k n", k=KC)
            z_v = cur[:, cj, XS_B:ROW].bitcast(BF16)

            q_ps = ps_q.tile([128, NT], F32, tag="q")
            for k in range(KC):
                nc.tensor.matmul(q_ps[:], wall[:, k, :], xs_v[:, k, :],
                                 start=(k == 0), stop=False)
            nc.tensor.matmul(q_ps[:], wall[:, 4, :], z_v,
                             start=False, stop=True)
            r_ps = ps_r.tile([128, NT], F32, tag="r")
            for k in range(KC):
                nc.tensor.matmul(r_ps[:], wall[:, 5 + k, :], xs_v[:, k, :],
                                 start=(k == 0), stop=False)
            nc.tensor.matmul(r_ps[:], wall[:, 9, :], z_v,
                             start=False, stop=True)

            if carry is not None:
                finish(carry)
                carry = None

            zt_ps = ps_zt.tile([128, NT], F32, tag="zt")
            for k in range(KC):
                nc.tensor.matmul(zt_ps[:], wall[:, 10 + k, :], xs_v[:, k, :],
                                 start=(k == 0), stop=False)

            q_s = sig.tile([128, NT], BF16, tag="q_s")
            nc.scalar.activation(q_s[:], q_ps[:],
                                 mybir.ActivationFunctionType.Sigmoid,
                                 bias=ball[:, 0:1])
            r_s = acts.tile([128, NT], BF16, tag="r_s")
            nc.scalar.activation(r_s[:], r_ps[:],
                                 mybir.ActivationFunctionType.Sigmoid,
                                 bias=ball[:, 1:2])
            rz = acts.tile([128, NT], BF16, tag="rz")
            nc.vector.tensor_mul(rz[:], r_s[:], z_v)

            carry = (zt_ps, q_s, rz, z_v, t)

        finish(carry)

    nc.compile()
    _module_cache[key] = nc
    return nc


def _pack_inputs(xs, zp):
    """Per-core [128, T, ROW] uint8 rows: 4096B bf16 xs + 1024B bf16 z."""
    xsb = xs.astype(ml_dtypes.bfloat16)
    v = xsb.reshape(B, KC, 128, N_LT, NT)        # [b, k, p, i, n]
    v = np.ascontiguousarray(v.transpose(0, 2, 3, 1, 4))  # [b, p, i, k, n]
    v = v.reshape(B, 128, N_LT, KC * NT).view(np.uint8)   # [b, p, i, XS_B]
    zb = zp.astype(ml_dtypes.bfloat16).reshape(B, 128, N_LT, NT)
    zb = zb.view(np.uint8).reshape(B, 128, N_LT, Z_B)
    rows = np.concatenate([v, zb], axis=-1)      # [b, p, i, ROW]
    cores = []
    for c in range(N_CORES):
        rc = rows[c * B_PER:(c + 1) * B_PER]     # [B_PER, p, i, ROW]
        rc = rc.transpose(1, 0, 2, 3).reshape(128, T, ROW)
        cores.append(np.ascontiguousarray(rc))
    return cores


def _pack_weights(inputs):
    wall = np.zeros((128, 15, 128), dtype=np.float32)
    ball = np.zeros((128, 3), dtype=np.float32)
    for g, (wn, un, wbn, ubn) in enumerate((
            ("Wq_w", "Uq_w", "Wq_b", "Uq_b"),
            ("Wr_w", "Ur_w", "Wr_b", "Ur_b"),
            ("Wz_w", "Uz_w", "Wz_b", "Uz_b"))):
        W = np.asarray(inputs[wn], dtype=np.float32)         # [128, 512]
        U = np.asarray(inputs[un], dtype=np.float32)         # [128, 128]
        Wt = W.T.reshape(KC, 128, 128)                       # [k, p, m]
        for k in range(KC):
            wall[:, 5 * g + k, :] = Wt[k]
        wall[:, 5 * g + 4, :] = U.T
        ball[:, g] = (np.asarray(inputs[wbn], dtype=np.float32)
                      + np.asarray(inputs[ubn], dtype=np.float32))
    wall_b = np.ascontiguousarray(wall.astype(ml_dtypes.bfloat16))
    return wall_b, np.ascontiguousarray(ball)


def _run(inputs, trace=False, **run_kwargs):
    xs = np.asarray(inputs["xs"], dtype=np.float32)
    zp = np.ascontiguousarray(np.asarray(inputs["z_prev"], dtype=np.float32))
    assert xs.shape == (B, IN_DIM, L) and zp.shape == (B, WIDTH, L)

    inp_cores = _pack_inputs(xs, zp)
    wall, ball = _pack_weights(inputs)

    nc = _build()
    in_maps = [{"inp": inp_cores[c], "wall": wall, "ball": ball}
               for c in range(N_CORES)]

    res = run_bass_kernel_spmd(nc, in_maps, core_ids=list(range(N_CORES)),
                               trace=trace, **run_kwargs)
    outs = []
    for c in range(N_CORES):
        o = res.results[c]["out"]                 # [128, T, NT] bf16
        o = np.asarray(o).reshape(128, B_PER, N_LT, NT)
        o = o.transpose(1, 0, 2, 3).reshape(B_PER, 128, L)
        outs.append(o.astype(np.float32))
    out = np.concatenate(outs, axis=0)
    return out, res


def kernel(**inputs):
    out, _ = _run(inputs, trace=False)
    return out
